# revision 2
# baseline (speedup 1.0000x reference)
"""Distributed kNN retrieval kernel for Trainium2 (8 NeuronCores), v2.

Design (pool-sharded distributed kNN):
  - The 200000-row embedding pool is split row-wise into 8 shards of 25000
    (24 "pairs" of 1024 cols + one 424-col tail), one shard per NeuronCore.
  - Each core computes scores = queries @ shard.T in fp8 (e4m3) with
    perf_mode=DoubleRow (256-deep contraction per pass, 4 passes for K=1024,
    fp32 accumulate in PSUM). Per (pair, query-batch): 8 matmuls fill a
    [128, 1024] PSUM tile (2 banks).
  - Selection: one DVE tensor_reduce(max) per PSUM tile folds 4 columns
    into one bf16 slot value straight out of PSUM (transposed AP view, no
    intermediate copies). Member cols of slot (pair, s) are
    {pair*1024 + s + 256j, j=0..3} (tail: {24576 + s + 106j}). ALL 6250
    slot values per query per core are DMA'd out -- no on-device top-k.
    ACT and GpSimd stay near-idle (DMA triggers only); PE is the only
    loaded engine and runs gap-free at its fp8-DoubleRow streaming rate.
  - The host merges 8*6250 = 50000 slots per query, takes the top 256 by
    device slot score, expands each into its 4 candidate rows, re-scores
    them with an exact software emulation of XLA:CPU's f32 dot kernel
    (two sequential-FMA chunks of 512), sorts, takes top-128, gathers the
    embedding rows and applies the k_predicted mask.

The host re-scoring makes the final ordering bit-identical to the
reference's jnp.dot scores. Since every true top-128 row's slot must rank
within the global top-256 slots (each higher slot attests at least one
higher-scoring row), recall is strictly better than a per-slice top-8
scheme at the same TOPC.
"""

import numpy as np

POOL = 200000
D = 1024
MAXK = 128
NQ = 1024
NSH = 8              # shards / cores
SHW = 25000          # rows per shard (no padding)
NPAIR = 25           # 24 full 1024-col pairs + one 424-col tail
TAILW = SHW - 24 * 1024   # 424
TAILQ = TAILW // 4        # 106
NB = 8               # query batches (1024 / 128)
KP = 4               # contraction passes (1024 / 256, DoubleRow)
NSLOT = SHW // 4     # 6250 slot values per query per core
TOPC = 256           # candidate slots (x4 rows) re-scored exactly per query
ESCALE = 64.0        # emb pre-scale so fp8 values are normal-range

_cache = {}


def _build():
    import concourse.tile as tile
    from concourse import bacc, mybir
    from contextlib import ExitStack

    DR = mybir.MatmulPerfMode.DoubleRow
    nc = bacc.Bacc("TRN2", target_bir_lowering=False, debug=False)
    qT = nc.dram_tensor("qT", [D, NQ], mybir.dt.float8e4, kind="ExternalInput").ap()
    embT = nc.dram_tensor("embT", [D, SHW], mybir.dt.float8e4, kind="ExternalInput").ap()
    cand_v = nc.dram_tensor("cand_v", [NQ, NSLOT], mybir.dt.bfloat16, kind="ExternalOutput").ap()

    with tile.TileContext(nc) as tc:
        with ExitStack() as ctx:
            qpool = ctx.enter_context(tc.tile_pool(name="q", bufs=1))
            epool = ctx.enter_context(tc.tile_pool(name="e", bufs=3))
            o2pool = ctx.enter_context(tc.tile_pool(name="o2", bufs=4))
            pspool = ctx.enter_context(tc.tile_pool(name="ps", bufs=4, space="PSUM"))

            # resident query tiles: per 256-deep pass [128, 2, 1024] (all batches)
            qts = []
            for p in range(KP):
                qt = qpool.tile([128, 2, NQ], mybir.dt.float8e4, tag=f"qt{p}")
                qts.append(qt)
            wq = qpool.tile([128, 2, 128], mybir.dt.float8e4, tag="wq")
            we = qpool.tile([128, 2, 512], mybir.dt.float8e4, tag="we")

            def load_pair(pair, engs):
                w = min(1024, SHW - pair * 1024)
                et = epool.tile([128, KP, 2, 1024], mybir.dt.float8e4, tag="et")
                for p in range(KP):
                    for i in range(2):
                        r = p * 256 + i * 128
                        engs[(2 * p + i) % len(engs)].dma_start(
                            et[:, p, i, :w],
                            embT[r:r + 128, pair * 1024:pair * 1024 + w])
                return et

            # PE warmup on zeros while the input DMAs are in flight
            nc.gpsimd.memset(wq[:], 0)
            nc.gpsimd.memset(we[:], 0)
            wps = pspool.tile([128, 1024], mybir.dt.float32, tag="ps")
            for _ in range(12):
                nc.tensor.matmul(wps[:, 0:512], wq[:], we[:],
                                 start=True, stop=True, perf_mode=DR)

            # startup: spread the first unit's dependencies over the 3 queues
            nc.sync.dma_start(qts[0][:, 0, :], qT[0:128, :])
            nc.sync.dma_start(qts[0][:, 1, :], qT[128:256, :])
            order = list(range(NPAIR))
            ets = {order[0]: load_pair(order[0], [nc.scalar, nc.gpsimd])}
            for p in range(1, KP):
                for i in range(2):
                    r = p * 256 + i * 128
                    nc.sync.dma_start(qts[p][:, i, :], qT[r:r + 128, :])
            ets[order[1]] = load_pair(order[1], [nc.sync, nc.scalar])

            for oi, pair in enumerate(order):
                if oi + 2 < NPAIR:
                    engs = [nc.sync, nc.scalar] if oi % 2 == 0 else [nc.scalar, nc.sync]
                    ets[order[oi + 2]] = load_pair(order[oi + 2], engs)
                et = ets.pop(pair)
                full = pair < NPAIR - 1
                fq = 256 if full else TAILQ
                o2 = o2pool.tile([128, NB, 256], mybir.dt.bfloat16, tag="o2")
                w = 256 if full else TAILQ
                for b in range(NB):
                    # flat ps: col c of pair <-> j = c//fq, s = c%fq;
                    # slot s members {pair*1024 + s + fq*j}
                    ps = pspool.tile([128, 1024], mybir.dt.float32, tag="ps")
                    if full:
                        for h in range(2):
                            for p in range(KP):
                                nc.tensor.matmul(
                                    ps[:, h * 512:(h + 1) * 512],
                                    qts[p][:, :, b * 128:(b + 1) * 128],
                                    et[:, p, :, h * 512:(h + 1) * 512],
                                    start=(p == 0), stop=(p == KP - 1),
                                    perf_mode=DR,
                                )
                    else:
                        for p in range(KP):
                            nc.tensor.matmul(
                                ps[:, 0:TAILW],
                                qts[p][:, :, b * 128:(b + 1) * 128],
                                et[:, p, :, 0:TAILW],
                                start=(p == 0), stop=(p == KP - 1),
                                perf_mode=DR,
                            )
                    nc.vector.tensor_reduce(
                        o2[:, b, 0:w],
                        ps[:, 0:4 * w].rearrange("p (j s) -> p s j", j=4),
                        axis=mybir.AxisListType.X, op=mybir.AluOpType.max)
                    oeng = nc.gpsimd if b % 2 == 0 else nc.scalar
                    oeng.dma_start(
                        cand_v[b * 128:(b + 1) * 128,
                               pair * 256:pair * 256 + w],
                        o2[:, b, 0:w])
    nc.compile()
    return nc


def _get_nc():
    if "nc" not in _cache:
        _cache["nc"] = _build()
    return _cache["nc"]


def _exact_rescore(q_rows, e_rows):
    """Bit-exact emulation of XLA:CPU f32 dot for K=1024: two sequential-FMA
    chunks of 512 (fp64 products+adds rounded to fp32 each step = fused
    multiply-add up to negligible double-rounding), summed in fp32."""
    a = q_rows.astype(np.float64)
    b = e_rows.astype(np.float64)
    out = np.zeros(len(a), np.float32)
    for c in range(2):
        acc = np.zeros(len(a), np.float32)
        for k in range(c * 512, (c + 1) * 512):
            acc = (a[:, k] * b[:, k] + acc).astype(np.float32)
        out = (out + acc).astype(np.float32)
    return out


def _install_ntff_hook():
    """The image's antenv lacks axon_hooks; synthesize it so trace=True works."""
    import sys, types
    if "antenv.axon_hooks" in sys.modules:
        return
    try:
        from trn_agent_boot.trn_boot import _ntff_profile_via_ctypes
        hook = _ntff_profile_via_ctypes("/opt/axon/libaxon_pjrt.so")
    except Exception:
        hook = None
    mod = types.ModuleType("antenv.axon_hooks")
    mod._hook = hook
    mod.get_axon_ntff_profile_hook = lambda: mod._hook
    mod.set_axon_ntff_profile_hook = lambda h: setattr(mod, "_hook", h)
    sys.modules["antenv.axon_hooks"] = mod


def _run_device(qT, shards, trace=False, tmpdir=None):
    import time
    from concourse.bass_utils import run_bass_kernel_spmd
    if trace:
        _install_ntff_hook()
    nc = _get_nc()
    in_maps = [{"qT": qT, "embT": shT} for shT in shards]
    last = None
    for attempt in range(3):
        try:
            return run_bass_kernel_spmd(nc, in_maps, list(range(NSH)), trace=trace, tmpdir=tmpdir)
        except Exception as e:  # transient device wedge: back off and retry
            last = e
            time.sleep(5 * (attempt + 1))
    raise last


def kernel(query_hidden, embeddings, k_predicted, phase_idx=None, _trace=False, _tmpdir=None):
    batch, seq, dim = query_hidden.shape
    q = np.ascontiguousarray(np.asarray(query_hidden, dtype=np.float32).reshape(-1, dim))
    emb = np.ascontiguousarray(np.asarray(embeddings, dtype=np.float32))
    nq = q.shape[0]
    assert (nq, dim) == (NQ, D) and emb.shape == (POOL, D)

    import ml_dtypes
    f8 = np.dtype(ml_dtypes.float8_e4m3)
    qT = np.ascontiguousarray(np.clip(q.T, -240, 240).astype(f8))
    shards = [
        np.ascontiguousarray(
            np.clip(emb[s * SHW:(s + 1) * SHW].T * ESCALE, -240, 240).astype(f8))
        for s in range(NSH)
    ]

    res = _run_device(qT, shards, trace=_trace, tmpdir=_tmpdir)
    _cache["last_res"] = res

    vals = np.stack([np.asarray(res.results[s]["cand_v"], np.float32)
                     for s in range(NSH)], 0)                           # [8, NQ, 6250]
    vals = np.transpose(vals, (1, 0, 2)).reshape(NQ, -1)                # [NQ, 50000]

    # top-TOPC slots by device score per query
    part = np.argpartition(-vals, TOPC, axis=1)[:, :TOPC]               # [NQ, TOPC]
    shard_of = part // NSLOT
    slot = part % NSLOT
    pair = slot // 256
    off = slot % 256
    fq = np.where(pair < NPAIR - 1, 256, TAILQ)

    # expand each slot into its 4 fold members (cols base + off + fq*j)
    loc4 = (pair * 1024 + off)[:, :, None] + fq[:, :, None] * np.arange(4, dtype=np.int64)[None, None, :]
    valid = loc4 < SHW
    cidx = (shard_of[:, :, None] * SHW + np.minimum(loc4, SHW - 1)).reshape(NQ, -1)
    valid = valid.reshape(NQ, -1)

    # exact re-score (bit-identical to the reference's jnp.dot)
    NC4 = 4 * TOPC
    flat_q = np.repeat(np.arange(NQ), NC4)
    flat_e = cidx.reshape(-1)
    exact = np.empty(NQ * NC4, np.float32)
    CH = 262144
    for o in range(0, NQ * NC4, CH):
        exact[o:o + CH] = _exact_rescore(q[flat_q[o:o + CH]], emb[flat_e[o:o + CH]])
    exact = exact.reshape(NQ, NC4)
    exact[~valid] = -np.inf

    # reference ordering: descending score, ties -> lower index first
    order = np.lexsort((cidx, -exact.astype(np.float64)), axis=1)[:, :MAXK]
    top_idx = np.take_along_axis(cidx, order, 1)                        # [NQ, 128]

    kp = np.asarray(k_predicted).reshape(-1)
    mask = (np.arange(MAXK)[None, :] < kp[:, None]).astype(np.float32)
    out = emb[top_idx] * mask[:, :, None]
    return out.reshape(batch, seq, MAXK, dim).astype(np.float32)


# revision 3
# speedup vs baseline: 1.0137x; 1.0137x over previous
"""Distributed kNN retrieval kernel for Trainium2 (8 NeuronCores), v2.

Design (pool-sharded distributed kNN):
  - The 200000-row embedding pool is split row-wise into 8 shards of 25000
    (24 "pairs" of 1024 cols + one 424-col tail), one shard per NeuronCore.
  - Each core computes scores = queries @ shard.T in fp8 (e4m3) with
    perf_mode=DoubleRow (256-deep contraction per pass, 4 passes for K=1024,
    fp32 accumulate in PSUM). Per (pair, query-batch): 8 matmuls fill a
    [128, 1024] PSUM tile (2 banks).
  - Selection: one DVE tensor_reduce(max) per PSUM tile folds 4 columns
    into one bf16 slot value straight out of PSUM (transposed AP view, no
    intermediate copies). Member cols of slot (pair, s) are
    {pair*1024 + s + 256j, j=0..3} (tail: {24576 + s + 106j}). ALL 6250
    slot values per query per core are DMA'd out -- no on-device top-k.
    ACT and GpSimd stay near-idle (DMA triggers only); PE is the only
    loaded engine and runs gap-free at its fp8-DoubleRow streaming rate.
  - The host merges 8*6250 = 50000 slots per query, takes the top 256 by
    device slot score, expands each into its 4 candidate rows, re-scores
    them with an exact software emulation of XLA:CPU's f32 dot kernel
    (two sequential-FMA chunks of 512), sorts, takes top-128, gathers the
    embedding rows and applies the k_predicted mask.

The host re-scoring makes the final ordering bit-identical to the
reference's jnp.dot scores. Since every true top-128 row's slot must rank
within the global top-256 slots (each higher slot attests at least one
higher-scoring row), recall is strictly better than a per-slice top-8
scheme at the same TOPC.
"""

import numpy as np

POOL = 200000
D = 1024
MAXK = 128
NQ = 1024
NSH = 8              # shards / cores
SHW = 25000          # rows per shard (no padding)
NPAIR = 25           # 24 full 1024-col pairs + one 424-col tail
TAILW = SHW - 24 * 1024   # 424
TAILQ = TAILW // 4        # 106
NB = 8               # query batches (1024 / 128)
KP = 4               # contraction passes (1024 / 256, DoubleRow)
NSLOT = SHW // 4     # 6250 slot values per query per core
TOPC = 256           # candidate slots (x4 rows) re-scored exactly per query
ESCALE = 64.0        # emb pre-scale so fp8 values are normal-range

_cache = {}


def _build():
    import concourse.tile as tile
    from concourse import bacc, mybir
    from contextlib import ExitStack

    DR = mybir.MatmulPerfMode.DoubleRow
    nc = bacc.Bacc("TRN2", target_bir_lowering=False, debug=False)
    qT = nc.dram_tensor("qT", [D, NQ], mybir.dt.float8e4, kind="ExternalInput").ap()
    embT = nc.dram_tensor("embT", [D, SHW], mybir.dt.float8e4, kind="ExternalInput").ap()
    cand_v = nc.dram_tensor("cand_v", [NQ, NSLOT], mybir.dt.bfloat16, kind="ExternalOutput").ap()

    with tile.TileContext(nc) as tc:
        with ExitStack() as ctx:
            qpool = ctx.enter_context(tc.tile_pool(name="q", bufs=1))
            epool = ctx.enter_context(tc.tile_pool(name="e", bufs=3))
            o2pool = ctx.enter_context(tc.tile_pool(name="o2", bufs=4))
            pspool = ctx.enter_context(tc.tile_pool(name="ps", bufs=4, space="PSUM"))

            # resident query tiles: per 256-deep pass [128, 2, 1024] (all batches)
            qts = []
            for p in range(KP):
                qt = qpool.tile([128, 2, NQ], mybir.dt.float8e4, tag=f"qt{p}")
                qts.append(qt)
            wq = qpool.tile([128, 2, 128], mybir.dt.float8e4, tag="wq")
            we = qpool.tile([128, 2, 512], mybir.dt.float8e4, tag="we")

            def load_pair(pair, engs):
                w = min(1024, SHW - pair * 1024)
                et = epool.tile([128, KP, 2, 1024], mybir.dt.float8e4, tag="et")
                for p in range(KP):
                    for i in range(2):
                        r = p * 256 + i * 128
                        engs[(2 * p + i) % len(engs)].dma_start(
                            et[:, p, i, :w],
                            embT[r:r + 128, pair * 1024:pair * 1024 + w])
                return et

            # PE warmup on zeros while the input DMAs are in flight
            nc.gpsimd.memset(wq[:], 0)
            nc.gpsimd.memset(we[:], 0)
            wps = pspool.tile([128, 1024], mybir.dt.float32, tag="ps")
            for _ in range(12):
                nc.tensor.matmul(wps[:, 0:512], wq[:], we[:],
                                 start=True, stop=True, perf_mode=DR)

            # startup: spread the first unit's dependencies over the 3 queues
            nc.sync.dma_start(qts[0][:, 0, :], qT[0:128, :])
            nc.sync.dma_start(qts[0][:, 1, :], qT[128:256, :])
            order = list(range(NPAIR))
            ets = {order[0]: load_pair(order[0], [nc.scalar, nc.gpsimd])}
            for p in range(1, KP):
                for i in range(2):
                    r = p * 256 + i * 128
                    nc.sync.dma_start(qts[p][:, i, :], qT[r:r + 128, :])
            ets[order[1]] = load_pair(order[1], [nc.sync, nc.scalar])

            for oi, pair in enumerate(order):
                if oi + 2 < NPAIR:
                    engs = [nc.sync, nc.scalar] if oi % 2 == 0 else [nc.scalar, nc.sync]
                    ets[order[oi + 2]] = load_pair(order[oi + 2], engs)
                et = ets.pop(pair)
                full = pair < NPAIR - 1
                fq = 256 if full else TAILQ
                o2 = o2pool.tile([128, NB, 256], mybir.dt.bfloat16, tag="o2")
                w = 256 if full else TAILQ
                for b in range(NB):
                    # flat ps: col c of pair <-> j = c//fq, s = c%fq;
                    # slot s members {pair*1024 + s + fq*j}
                    ps = pspool.tile([128, 1024], mybir.dt.float32, tag="ps")
                    if full:
                        for h in range(2):
                            for p in range(KP):
                                nc.tensor.matmul(
                                    ps[:, h * 512:(h + 1) * 512],
                                    qts[p][:, :, b * 128:(b + 1) * 128],
                                    et[:, p, :, h * 512:(h + 1) * 512],
                                    start=(p == 0), stop=(p == KP - 1),
                                    perf_mode=DR,
                                )
                    else:
                        for p in range(KP):
                            nc.tensor.matmul(
                                ps[:, 0:TAILW],
                                qts[p][:, :, b * 128:(b + 1) * 128],
                                et[:, p, :, 0:TAILW],
                                start=(p == 0), stop=(p == KP - 1),
                                perf_mode=DR,
                            )
                    nc.vector.tensor_reduce(
                        o2[:, b, 0:w],
                        ps[:, 0:4 * w].rearrange("p (j s) -> p s j", j=4),
                        axis=mybir.AxisListType.X, op=mybir.AluOpType.max)
                # one output DMA per pair (all 8 batches), alternating queues
                oeng = nc.scalar if pair % 2 == 0 else nc.sync
                dst = cand_v[0:NQ, pair * 256:pair * 256 + w].rearrange(
                    "(b p) s -> p b s", b=NB)
                oeng.dma_start(dst, o2[:, :, 0:w])
    nc.compile()
    return nc


def _get_nc():
    if "nc" not in _cache:
        _cache["nc"] = _build()
    return _cache["nc"]


def _exact_rescore(q_rows, e_rows):
    """Bit-exact emulation of XLA:CPU f32 dot for K=1024: two sequential-FMA
    chunks of 512 (fp64 products+adds rounded to fp32 each step = fused
    multiply-add up to negligible double-rounding), summed in fp32."""
    a = q_rows.astype(np.float64)
    b = e_rows.astype(np.float64)
    out = np.zeros(len(a), np.float32)
    for c in range(2):
        acc = np.zeros(len(a), np.float32)
        for k in range(c * 512, (c + 1) * 512):
            acc = (a[:, k] * b[:, k] + acc).astype(np.float32)
        out = (out + acc).astype(np.float32)
    return out


def _install_ntff_hook():
    """The image's antenv lacks axon_hooks; synthesize it so trace=True works."""
    import sys, types
    if "antenv.axon_hooks" in sys.modules:
        return
    try:
        from trn_agent_boot.trn_boot import _ntff_profile_via_ctypes
        hook = _ntff_profile_via_ctypes("/opt/axon/libaxon_pjrt.so")
    except Exception:
        hook = None
    mod = types.ModuleType("antenv.axon_hooks")
    mod._hook = hook
    mod.get_axon_ntff_profile_hook = lambda: mod._hook
    mod.set_axon_ntff_profile_hook = lambda h: setattr(mod, "_hook", h)
    sys.modules["antenv.axon_hooks"] = mod


def _run_device(qT, shards, trace=False, tmpdir=None):
    import time
    from concourse.bass_utils import run_bass_kernel_spmd
    if trace:
        _install_ntff_hook()
    nc = _get_nc()
    in_maps = [{"qT": qT, "embT": shT} for shT in shards]
    last = None
    for attempt in range(3):
        try:
            return run_bass_kernel_spmd(nc, in_maps, list(range(NSH)), trace=trace, tmpdir=tmpdir)
        except Exception as e:  # transient device wedge: back off and retry
            last = e
            time.sleep(5 * (attempt + 1))
    raise last


def kernel(query_hidden, embeddings, k_predicted, phase_idx=None, _trace=False, _tmpdir=None):
    batch, seq, dim = query_hidden.shape
    q = np.ascontiguousarray(np.asarray(query_hidden, dtype=np.float32).reshape(-1, dim))
    emb = np.ascontiguousarray(np.asarray(embeddings, dtype=np.float32))
    nq = q.shape[0]
    assert (nq, dim) == (NQ, D) and emb.shape == (POOL, D)

    import ml_dtypes
    f8 = np.dtype(ml_dtypes.float8_e4m3)
    qT = np.ascontiguousarray(np.clip(q.T, -240, 240).astype(f8))
    shards = [
        np.ascontiguousarray(
            np.clip(emb[s * SHW:(s + 1) * SHW].T * ESCALE, -240, 240).astype(f8))
        for s in range(NSH)
    ]

    res = _run_device(qT, shards, trace=_trace, tmpdir=_tmpdir)
    _cache["last_res"] = res

    vals = np.stack([np.asarray(res.results[s]["cand_v"], np.float32)
                     for s in range(NSH)], 0)                           # [8, NQ, 6250]
    vals = np.transpose(vals, (1, 0, 2)).reshape(NQ, -1)                # [NQ, 50000]

    # top-TOPC slots by device score per query
    part = np.argpartition(-vals, TOPC, axis=1)[:, :TOPC]               # [NQ, TOPC]
    shard_of = part // NSLOT
    slot = part % NSLOT
    pair = slot // 256
    off = slot % 256
    fq = np.where(pair < NPAIR - 1, 256, TAILQ)

    # expand each slot into its 4 fold members (cols base + off + fq*j)
    loc4 = (pair * 1024 + off)[:, :, None] + fq[:, :, None] * np.arange(4, dtype=np.int64)[None, None, :]
    valid = loc4 < SHW
    cidx = (shard_of[:, :, None] * SHW + np.minimum(loc4, SHW - 1)).reshape(NQ, -1)
    valid = valid.reshape(NQ, -1)

    # exact re-score (bit-identical to the reference's jnp.dot)
    NC4 = 4 * TOPC
    flat_q = np.repeat(np.arange(NQ), NC4)
    flat_e = cidx.reshape(-1)
    exact = np.empty(NQ * NC4, np.float32)
    CH = 262144
    for o in range(0, NQ * NC4, CH):
        exact[o:o + CH] = _exact_rescore(q[flat_q[o:o + CH]], emb[flat_e[o:o + CH]])
    exact = exact.reshape(NQ, NC4)
    exact[~valid] = -np.inf

    # reference ordering: descending score, ties -> lower index first
    order = np.lexsort((cidx, -exact.astype(np.float64)), axis=1)[:, :MAXK]
    top_idx = np.take_along_axis(cidx, order, 1)                        # [NQ, 128]

    kp = np.asarray(k_predicted).reshape(-1)
    mask = (np.arange(MAXK)[None, :] < kp[:, None]).astype(np.float32)
    out = emb[top_idx] * mask[:, :, None]
    return out.reshape(batch, seq, MAXK, dim).astype(np.float32)
